# revision 7
# baseline (speedup 1.0000x reference)
"""Trainium2 Bass kernel for nn_CustomGINConv (gnn_message_passing).

Reference computation (per path n, L=6 layers, C=128 channels):
    h[l]    = x[l] @ Wt[:C] + emb[idx[l]] @ Wt[C:] + bt
    prop[l] = h[l-1] + h[l+1]                (zero-padded along l)
    u[l]    = (1+eps) * x[l] + prop[l]
    out     = sum_l relu(u[l] @ W1 + b1) @ W2 + b2   -> [N, C]

Kernel strategy (shard N across 8 cores, feature-major on-chip layout):
  * Everything linear before the relu is folded host-side. With
    T = emb @ Wt[C:] + bt and s = 1+eps (the eps scales cancel):
      z1[l] = x[l] @ (s*W1) + (x[l-1]+x[l+1]) @ (Wt[:C] @ W1)
              + ohsum[l] @ (T @ W1) + b1
    where ohsum[l] = onehot(idx[l-1]) + onehot(idx[l+1]).
    Then out = (sum_l relu(z1[l])) @ W2 + L*b2 — the L-sum is taken
    BEFORE the W2 matmul, so W2 runs once per tile, not once per layer.
  * b1 is folded into the one-hot matmul: oh gets two constant 1/16 rows
    (102 total) whose tw1 rows are a hi/lo fp8 split of 16*b1.
  * The embedding gather is a one-hot matmul in fp8 DoubleRow perf mode
    (values {0, 1/16, 2/16}, exact) against a 16x-scaled (T @ W1) table;
    [51, 2, .] packing on both operands keeps the contraction exact.
  * relu+sum over L is fused scalar_tensor_tensor chains
    (out = max(z,0) + acc) split across ACT / DVE / GPSIMD, producing two
    partial sums a (l=0..2) and b (l=3..5); the W2 matmul accumulates
    W2^T a + W2^T b in one PSUM bank.
  * The W2 matmuls for tile i are emitted after tile i+1's z matmuls
    (1-tile software pipeline) so the PE never waits on the relu chain.
  * x path bf16 (KERNEL_X_BF16=0 for fp32), output bf16 (KERNEL_OUT_BF16=0
    for fp32); host upcasts.
Per-tile engine budget (M=512 cols): PE 20 matmuls ~3.6us, DMA ~3.7us,
DVE ~2.7us, ACT ~1.3us, GPSIMD ~0.7us -> DMA-roofline bound.
"""

import os
import sys

import numpy as np

sys.path.insert(0, "/opt/trn_rl_repo")

import ml_dtypes  # noqa: E402

import concourse.bass as bass  # noqa: E402
import concourse.tile as tile  # noqa: E402
from concourse import bacc, mybir  # noqa: E402
from concourse import bass_utils  # noqa: E402
from concourse.bass import ts  # noqa: E402

L = 6
N_FULL = 65536
C = 128
EMB = 100
EMB_B = EMB + 2  # +2 bias rows (hi/lo fp8 split of b1)
EMB_H = EMB_B // 2
NCORES = 8
NC_N = N_FULL // NCORES  # 8192 rows per core
M = 512  # tile width (columns of the feature-major layout)

F32 = mybir.dt.float32
F32R = mybir.dt.float32r
BF16 = mybir.dt.bfloat16
F8 = mybir.dt.float8e4

X_BF16 = os.environ.get("KERNEL_X_BF16", "1") == "1"
OUT_BF16 = os.environ.get("KERNEL_OUT_BF16", "1") == "1"
RELU = mybir.ActivationFunctionType.Relu
IDENT = mybir.ActivationFunctionType.Identity
ADD = mybir.AluOpType.add
MAX = mybir.AluOpType.max

# fp8e4m3 bit patterns for {0, 1/16, 2/16}: the one-hot carries a 1/16
# factor (exact powers of two) and tw1 is pre-scaled by 16 so its small
# entries sit in fp8's normal range instead of the subnormals.
_FP8_LUT = np.array([0x00, 0x18, 0x20], dtype=np.uint8)
_OH_SCALE = np.float32(16.0)


def build_bass(nc_n: int = NC_N, num_devices: int = NCORES,
               repeat: int = 1, bump: float = 1.0) -> bass.Bass:
    """Build + compile the per-core Bass program (same program on all cores).

    repeat>1 re-runs the whole tile loop (for timing: on-device work scales
    by `repeat` while dispatch overhead stays fixed)."""
    nc = bacc.Bacc(
        "TRN2",
        target_bir_lowering=False,
        debug=False,
        enable_asserts=False,
        num_devices=num_devices,
    )
    XDT = BF16 if X_BF16 else F32R
    ODT = BF16 if OUT_BF16 else F32
    nt = nc_n // M
    # tile-major DRAM layouts: each tile slice is contiguous per partition
    xt = nc.dram_tensor("xt", [C, nt, L, M], XDT, kind="ExternalInput").ap()
    oh = nc.dram_tensor("oh", [EMB_H, nt, 2, L, M], F8, kind="ExternalInput").ap()
    w1d = nc.dram_tensor("w1d", [C, C], XDT, kind="ExternalInput").ap()
    w1x = nc.dram_tensor("w1x", [C, C], XDT, kind="ExternalInput").ap()
    tw1 = nc.dram_tensor("tw1", [EMB_H, 2, C], F8, kind="ExternalInput").ap()
    w2 = nc.dram_tensor("w2", [C, C], F32R, kind="ExternalInput").ap()
    b2s = nc.dram_tensor("b2s", [C, 1], F32, kind="ExternalInput").ap()
    out = nc.dram_tensor("out", [C, nc_n], ODT, kind="ExternalOutput").ap()

    with tile.TileContext(nc) as tc:
        with (
            tc.tile_pool(name="consts", bufs=1) as consts,
            tc.tile_pool(name="xp", bufs=3) as xp,
            tc.tile_pool(name="ohp", bufs=3) as ohp,
            tc.tile_pool(name="zp", bufs=2) as zp,
            tc.tile_pool(name="outp", bufs=2) as outp,
            tc.tile_pool(name="pp", bufs=1, space="PSUM") as pp,
        ):
            w1d_sb = consts.tile([C, C], XDT, tag="w1d")
            nc.scalar.dma_start(w1d_sb[:], w1d)
            w1x_sb = consts.tile([C, C], XDT, tag="w1x")
            nc.scalar.dma_start(w1x_sb[:], w1x)
            tw1_sb = consts.tile([EMB_H, 2, C], F8, tag="tw1")
            nc.scalar.dma_start(tw1_sb[:], tw1)
            w2_sb = consts.tile([C, C], F32R, tag="w2")
            nc.scalar.dma_start(w2_sb[:], w2)
            b2_sb = consts.tile([C, 1], F32, tag="b2")
            nc.scalar.dma_start(b2_sb[:], b2s)

            def emit_y(prev):
                a_sb, b_sb, pi = prev
                y_ps = pp.tile([C, M], F32, tag="y", bufs=2)
                nc.tensor.matmul(y_ps[:], w2_sb[:], a_sb[:], start=True,
                                 stop=False)
                nc.tensor.matmul(y_ps[:], w2_sb[:], b_sb[:], start=False,
                                 stop=True)
                out_t = outp.tile([C, M], ODT, tag="out")
                nc.scalar.activation(out_t[:], y_ps[:], IDENT, bias=b2_sb[:],
                                     scale=bump)
                # out rides the tile's oh-queue (see balance note below)
                q_out = nc.scalar if pi % 2 == 0 else nc.sync
                q_out.dma_start(out[:, ts(pi, M)], out_t[:])

            prev = None
            for i_rep in range(repeat * nt):
                i = i_rep % nt
                # Only SP and Act have hardware DGE queues; alternate the
                # whole-tensor assignments per tile so each queue moves
                # (x + oh + out)/2 bytes per tile pair.
                q_x = nc.sync if i % 2 == 0 else nc.scalar
                q_oh = nc.scalar if i % 2 == 0 else nc.sync
                xt_t = xp.tile([C, L, M], XDT, tag="xt")
                oh_t = ohp.tile([EMB_H, 2, L, M], F8, tag="oh")
                if i_rep == 0:
                    # split the very first loads per layer so l=0's matmuls
                    # start as soon as x[0], x[1], ohsum[0] land instead of
                    # waiting for the full tile
                    for l in range(L):
                        q_x.dma_start(xt_t[:, l, :], xt[:, i, l, :])
                        q_oh.dma_start(oh_t[:, :, l, :], oh[:, i, :, l, :])
                else:
                    q_x.dma_start(xt_t[:], xt[:, i, :, :])
                    q_oh.dma_start(oh_t[:], oh[:, i, :, :, :])

                # xs[l] = x[l-1] + x[l+1] for interior l (one stacked DVE op);
                # boundary layers use the single neighbor directly.
                xs_t = xp.tile([C, L - 2, M], XDT, tag="xs")
                nc.vector.tensor_tensor(
                    xs_t[:], xt_t[:, 0 : L - 2, :], xt_t[:, 2:L, :], ADD
                )

                # z1[l] (pre-relu, incl b1) accumulates in its own PSUM bank:
                #   W1'^T x[l] + (Wtx@W1)^T (x[l-1]+x[l+1]) + (T@W1)^T ohsum[l]
                zs = []
                for l in range(L):
                    z_ps = pp.tile([C, M], F32, tag="z", bufs=6)
                    nc.tensor.matmul(
                        z_ps[:], w1d_sb[:], xt_t[:, l, :], start=True,
                        stop=False,
                    )
                    nbr = (
                        xt_t[:, 1, :] if l == 0
                        else xt_t[:, L - 2, :] if l == L - 1
                        else xs_t[:, l - 1, :]
                    )
                    nc.tensor.matmul(z_ps[:], w1x_sb[:], nbr, start=False,
                                     stop=False)
                    nc.tensor.matmul(
                        z_ps[:], tw1_sb[:], oh_t[:, :, l, :],
                        start=False, stop=True,
                        perf_mode=mybir.MatmulPerfMode.DoubleRow,
                    )
                    zs.append(z_ps)

                # W2 matmuls for the PREVIOUS tile: emitted after this tile's
                # z matmuls so the PE never waits on the relu chain.
                if prev is not None:
                    emit_y(prev)

                # relu+sum over L: a = sum relu(z[0..2]), b = sum relu(z[3..5])
                # fused (max(z,0) + acc) chains split across ACT/DVE/GPSIMD.
                r0 = zp.tile([C, M], F32R, tag="r0")
                nc.scalar.activation(r0[:], zs[0][:], RELU)
                a1 = zp.tile([C, M], F32R, tag="a1")
                nc.vector.scalar_tensor_tensor(a1[:], zs[1][:], 0.0, r0[:],
                                               MAX, ADD)
                a2 = zp.tile([C, M], F32R, tag="a2")
                nc.vector.scalar_tensor_tensor(a2[:], zs[2][:], 0.0, a1[:],
                                               MAX, ADD)
                r3 = zp.tile([C, M], F32R, tag="r3")
                nc.scalar.activation(r3[:], zs[3][:], RELU)
                b1t = zp.tile([C, M], F32R, tag="b1t")
                nc.vector.scalar_tensor_tensor(b1t[:], zs[4][:], 0.0, r3[:],
                                               MAX, ADD)
                # GPSIMD cannot read PSUM: relu z5 on ACT, then SBUF-only add
                r5 = zp.tile([C, M], F32R, tag="r5")
                nc.scalar.activation(r5[:], zs[5][:], RELU)
                b2t = zp.tile([C, M], F32R, tag="b2t")
                nc.gpsimd.tensor_tensor(b2t[:], b1t[:], r5[:], ADD)
                prev = (a2, b2t, i)
            emit_y(prev)

    nc.compile()
    return nc


def prep_host(x, atomic_type, emb, Wt, bt, eps, W1, b1, W2, b2, nc_n=NC_N,
              ncores=NCORES):
    """Host-side prep: fold eps into weights, build per-core input maps."""
    x = np.asarray(x, dtype=np.float32)
    idx = np.asarray(atomic_type).astype(np.int64)
    emb = np.asarray(emb, dtype=np.float32)
    Wt = np.asarray(Wt, dtype=np.float32)
    bt = np.asarray(bt, dtype=np.float32)
    W1 = np.asarray(W1, dtype=np.float32)
    b1 = np.asarray(b1, dtype=np.float32)
    W2 = np.asarray(W2, dtype=np.float32)
    b2 = np.asarray(b2, dtype=np.float32)
    scale = 1.0 + np.float32(np.asarray(eps).reshape(-1)[0])
    nt = nc_n // M

    # W1 folded through the propagate step (eps-scales cancel in the products):
    #   z1[l] = x[l] @ (scale*W1) + x[l+/-1] @ (Wt[:C] @ W1) + ohsum[l] @ (T @ W1)
    # with T = emb @ Wt[C:] + bt.
    T = (emb @ Wt[C:]) + bt  # [EMB, C]
    xdt = ml_dtypes.bfloat16 if X_BF16 else np.float32
    w1d = np.ascontiguousarray((W1 * scale).astype(xdt))
    w1x = np.ascontiguousarray(
        (Wt[:C].astype(np.float64) @ W1.astype(np.float64)).astype(xdt)
    )
    tw1b = np.empty((EMB_B, C), dtype=ml_dtypes.float8_e4m3)
    tw1b[:EMB] = (_OH_SCALE * (T.astype(np.float64) @ W1.astype(np.float64))
                  ).astype(ml_dtypes.float8_e4m3)
    # bias rows: hi/lo fp8 split of 16*b1, paired with constant 1/16 oh rows
    b1s = _OH_SCALE * b1
    tw1b[EMB] = b1s.astype(ml_dtypes.float8_e4m3)
    tw1b[EMB + 1] = (b1s - tw1b[EMB].astype(np.float32)).astype(
        ml_dtypes.float8_e4m3
    )
    tw1 = np.ascontiguousarray(tw1b.reshape(EMB_H, 2, C))
    w2s = np.ascontiguousarray(W2)
    b2s = np.ascontiguousarray((np.float32(L) * b2).reshape(C, 1))

    arange_emb = np.arange(EMB, dtype=idx.dtype)
    in_maps = []
    for k in range(ncores):
        n0 = k * nc_n
        xs = x[:, n0 : n0 + nc_n, :]  # [L, nc_n, C]
        xtk = np.ascontiguousarray(
            xs.transpose(2, 1, 0)  # [C, nc_n, L]
            .reshape(C, nt, M, L)
            .transpose(0, 1, 3, 2)  # [C, nt, L, M]
        ).astype(xdt)
        ii = idx[:, n0 : n0 + nc_n]  # [L, nc_n]
        ohb = (ii[:, None, :] == arange_emb[None, :, None]).view(np.uint8)
        ohs = np.zeros((L, EMB_B, nc_n), dtype=np.uint8)
        ohs[:-1, :EMB] += ohb[1:]
        ohs[1:, :EMB] += ohb[:-1]
        ohs[:, EMB:] = 1
        ohk = _FP8_LUT[ohs.transpose(1, 0, 2)]  # [EMB_B, L, nc_n] uint8 bits
        ohk = np.ascontiguousarray(
            ohk.reshape(EMB_H, 2, L, nt, M).transpose(0, 3, 1, 2, 4)
        ).view(ml_dtypes.float8_e4m3)  # [EMB_H, nt, 2, L, M]
        in_maps.append(
            {
                "xt": xtk,
                "oh": ohk,
                "w1d": w1d,
                "w1x": w1x,
                "tw1": tw1,
                "w2": w2s,
                "b2s": b2s,
            }
        )
    return in_maps


_COMPILED = {}


def get_compiled(nc_n=NC_N, num_devices=NCORES):
    key = (nc_n, num_devices)
    if key not in _COMPILED:
        _COMPILED[key] = build_bass(nc_n, num_devices)
    return _COMPILED[key]


def run_on_hw(in_maps, nc=None, trace=False, **kwargs):
    if nc is None:
        nc = get_compiled()
    return bass_utils.run_bass_kernel_spmd(
        nc, in_maps, core_ids=list(range(len(in_maps))), trace=trace, **kwargs
    )


def kernel(**inputs) -> np.ndarray:
    in_maps = prep_host(
        inputs["x"],
        inputs["atomic_type"],
        inputs["emb"],
        inputs["Wt"],
        inputs["bt"],
        inputs["eps"],
        inputs["W1"],
        inputs["b1"],
        inputs["W2"],
        inputs["b2"],
    )
    res = run_on_hw(in_maps)
    out = np.empty((N_FULL, C), dtype=np.float32)
    for k in range(NCORES):
        out[k * NC_N : (k + 1) * NC_N, :] = res.results[k]["out"].astype(
            np.float32
        ).T
    return out


if __name__ == "__main__":
    import reference  # only when run manually inside /root/problem

    inputs = {k: np.asarray(v) for k, v in reference.setup_inputs().items()}
    got = kernel(**inputs)
    want = np.asarray(reference.reference(**inputs))
    err = np.abs(got - want).max() / np.abs(want).max()
    print("rel err:", err)
